# revision 4
# baseline (speedup 1.0000x reference)
"""Trainium2 distributed kernel for nn_AnomalousTokenPerception.

reference:
    probs = sigmoid(mask_logits)            # (256, 1024, 1024)
    mean  = probs.mean((1,2)); std = probs.std((1,2), ddof=1)
    count = (probs > 0.5).sum((1,2))
    emb   = normalize([mean, std, count], dim=1)   # (256, 3)
    sim   = emb @ emb.T; avg = sim.mean(1)
    out   = sigmoid(avg - avg.mean())       # (256,)

Strategy: shard the 256 instances across 8 cores (32 each). Each core
streams its 128 MiB of logits once; the kernel is HBM-bound, so the
loop is built around keeping the (measured ~380 us/pass, i.e.
~350 GB/s/core) DMA stream saturated: single-issuer SP DMAs with an
explicit 4-deep prefetch pipeline. Per (128, 8192) instance tile:
  - ACT:  sigmoid -> bf16 probs, fused accum_out -> per-partition S1
  - DVE:  is_gt(x,0) -> fp8 b tile; probs *= probs in place (bf16)
  - PE:   ones-matmul folds stream probs^2 and b into (1,512) PSUMs,
          which DVE reduces to per-instance S2 / C
Counts stay exact (0/1 fp8, integer f32 sums). S1's per-partition sums
fold via one ones-matmul at the end. The tiny (1, 96) per-core stats
are AllGathered; every core runs the (256,)-sized epilogue using
avg_i = e_i . (sum_j e_j) / N to avoid the NxN matmul. The raw
per-core stats ride along in the last 96 output elements so tests can
verify the reductions exactly; kernel() returns just the 256.

Notes from HW bring-up (do not "simplify" these away):
  - tensor_tensor_reduce / tensor_scalar+accum_out pass CoreSim and the
    BIR verifier but hang or miscompute on real TRN2 - avoided.
  - nc.gpsimd (Pool) runs elementwise ops ~8 G elem/s - never used for
    per-element work.
  - Issuing DMAs from the ACT HWDGE queue is faster when ACT is idle
    but regresses under sigmoid load - SP-only issue.
"""
import sys

if "/opt/trn_rl_repo" not in sys.path:
    sys.path.insert(0, "/opt/trn_rl_repo")

import numpy as np

import concourse.bass as bass
import concourse.tile as tile
from concourse import bacc, mybir
from concourse import bass_utils

F32 = mybir.dt.float32
BF16 = mybir.dt.bfloat16
F8 = mybir.dt.float8e4
AF = mybir.ActivationFunctionType
ALU = mybir.AluOpType

NCORES = 8
N = 256                 # instances
NI = N // NCORES        # instances per core = 32
P = 128                 # SBUF partitions
FD = 8192               # free dim: one instance per tile
NELEM = 1024 * 1024     # elements per instance
EPS = 1e-12
LOGITS_BUFS = 4         # DMA prefetch depth


def build(reps=1, pair=False):
    # reps>1: wrap the main loop in a dynamic For_i (benchmark only) —
    # runtime repetition, immune to compile-time dead-code elimination.
    nc = bacc.Bacc("TRN2", target_bir_lowering=False, debug=False,
                   num_devices=NCORES)
    x = nc.dram_tensor("mask_logits", [NI, P, FD], F32, kind="ExternalInput")
    out = nc.dram_tensor("out", [1, N + 3 * NI], F32, kind="ExternalOutput")

    with tile.TileContext(nc) as tc:
        with tc.tile_pool(name="logits",
                          bufs=(2 if pair else LOGITS_BUFS)) as logits_pool, \
             tc.tile_pool(name="probsp", bufs=2) as probsp, \
             tc.tile_pool(name="bp", bufs=2) as bp, \
             tc.tile_pool(name="singles", bufs=1) as singles, \
             tc.tile_pool(name="psum", bufs=2, space="PSUM") as psum_pool, \
             tc.tile_pool(name="dram", bufs=1, space="DRAM") as dram:

            s1t = singles.tile([P, NI], F32)
            s2row = singles.tile([1, NI], F32)
            crow = singles.tile([1, NI], F32)
            ones = singles.tile([P, 1], F32)
            nc.vector.memset(ones[:], 1.0)
            onesr = singles.tile([P, 1], F8)
            nc.vector.memset(onesr[:], 1.0)
            onesb = singles.tile([P, 1], BF16)
            nc.vector.memset(onesb[:], 1.0)

            def fold(src_ap, dst, tag, w):
                # ones-matmul partial reduce (128,FD)->(1,512)->scalar
                ps = psum_pool.tile([1, 512], F32, name=tag, tag=tag)
                nchunk = FD // 512
                for k in range(nchunk):
                    nc.tensor.matmul(ps[:], w[:],
                                     src_ap[:, k * 512:(k + 1) * 512],
                                     start=(k == 0), stop=(k == nchunk - 1))
                nc.vector.tensor_reduce(out=dst, in_=ps[:],
                                        axis=mybir.AxisListType.X, op=ALU.add)

            def load(i):
                xt = logits_pool.tile([P, FD], F32, name="xt", tag="xt")
                nc.sync.dma_start(xt[:], x[i])
                return xt

            def load_pair(j):
                xt = logits_pool.tile([P, 2 * FD], F32, name="xt", tag="xt")
                src = x[j * 2:j * 2 + 2].rearrange("a p f -> p a f")
                nc.sync.dma_start(
                    xt[:].rearrange("p (a f) -> p a f", a=2), src)
                return xt

            def main_block():
                pf = 2 if pair else LOGITS_BUFS
                tiles = {}
                if pair:
                    for j in range(min(pf, NI // 2)):
                        tiles[j] = load_pair(j)
                else:
                    for i in range(min(pf, NI)):
                        tiles[i] = load(i)
                for i in range(NI):
                    if pair:
                        xp = tiles[i // 2]
                        xt = xp[:, (i % 2) * FD:(i % 2 + 1) * FD]
                        if i % 2 == 1:
                            del tiles[i // 2]
                    else:
                        xt = tiles.pop(i)
                    xta = xt if isinstance(xt, bass.AP) else xt[:]
                    probs = probsp.tile([P, FD], BF16, name="probs",
                                        tag="probs")
                    nc.scalar.activation(probs[:], xta, AF.Sigmoid,
                                         accum_out=s1t[:, i:i + 1])
                    b = bp.tile([P, FD], F8, name="b", tag="b")
                    nc.vector.tensor_scalar(
                        out=b[:], in0=xta, scalar1=0.0, scalar2=None,
                        op0=ALU.is_gt)
                    if pair:
                        j = i // 2
                        if i % 2 == 1 and j + pf < NI // 2:
                            tiles[j + pf] = load_pair(j + pf)
                    elif i + pf < NI:
                        tiles[i + pf] = load(i + pf)
                    # square probs in place, then fold both stats
                    nc.vector.tensor_tensor(out=probs[:], in0=probs[:],
                                            in1=probs[:], op=ALU.mult)
                    fold(probs[:], s2row[:, i:i + 1], "psq", onesb)
                    fold(b[:], crow[:, i:i + 1], "pcnt", onesr)

            if reps == 1:
                main_block()
            else:
                with tc.For_i(0, reps, 1):
                    main_block()

            # fold S1 partitions via ones matmul -> PSUM (1, NI)
            folded = psum_pool.tile([1, NI], F32)
            nc.tensor.matmul(folded[:], ones[:], s1t[:])

            # comb (1, 96) = [S1 | S2 | C]
            comb = singles.tile([1, 3 * NI], F32)
            nc.vector.tensor_copy(comb[:, 0:NI], folded[:])
            nc.vector.tensor_copy(comb[:, NI:2 * NI], s2row[:])
            nc.vector.tensor_copy(comb[:, 2 * NI:3 * NI], crow[:])

            # all-gather per-core (1, 96) stats -> (8, 96)
            cc_in = dram.tile([1, 3 * NI], F32)
            cc_out = dram.tile([NCORES, 3 * NI], F32)
            nc.sync.dma_start(cc_in[:], comb[:])
            nc.gpsimd.collective_compute(
                "AllGather", ALU.bypass,
                replica_groups=[list(range(NCORES))],
                ins=[cc_in[:].opt()], outs=[cc_out[:].opt()])

            # epilogue on partition 0, N lanes.
            # single rearranged DMA gathers all three stat rows:
            # rows3[0, k*N + c*NI + j] = cc_out[c, k*NI + j]
            rows3 = singles.tile([1, 3 * N], F32)
            nc.sync.dma_start(
                rows3[:].rearrange("p (k c j) -> p k c j", k=3, c=NCORES),
                cc_out[:].rearrange("c (k j) -> k c j", k=3)[None])
            s1r = rows3[:, 0:N]
            s2r = rows3[:, N:2 * N]
            cr = rows3[:, 2 * N:3 * N]

            _row_n = [0]

            def row_tile():
                _row_n[0] += 1
                return singles.tile([1, N], F32, name=f"row{_row_n[0]}",
                                    tag=f"row{_row_n[0]}")

            n = float(NELEM)
            mean = row_tile()
            nc.vector.tensor_scalar_mul(out=mean[:], in0=s1r,
                                        scalar1=1.0 / n)
            t1 = row_tile()
            nc.vector.tensor_tensor(out=t1[:], in0=s1r, in1=mean[:],
                                    op=ALU.mult)
            var = row_tile()
            nc.vector.tensor_tensor(out=var[:], in0=s2r, in1=t1[:],
                                    op=ALU.subtract)
            nc.vector.tensor_scalar_mul(out=var[:], in0=var[:],
                                        scalar1=1.0 / (n - 1.0))
            std = row_tile()
            nc.scalar.activation(std[:], var[:], AF.Sqrt)

            # ||e||^2 = mean^2 + var + count^2   (std^2 == var)
            nsq = row_tile()
            nc.vector.tensor_tensor(out=nsq[:], in0=mean[:], in1=mean[:],
                                    op=ALU.mult)
            nc.vector.tensor_tensor(out=nsq[:], in0=nsq[:], in1=var[:],
                                    op=ALU.add)
            tmp = row_tile()
            nc.vector.tensor_tensor(out=tmp[:], in0=cr, in1=cr,
                                    op=ALU.mult)
            nc.vector.tensor_tensor(out=nsq[:], in0=nsq[:], in1=tmp[:],
                                    op=ALU.add)
            norm = row_tile()
            nc.scalar.activation(norm[:], nsq[:], AF.Sqrt)
            nc.vector.tensor_scalar_max(out=norm[:], in0=norm[:], scalar1=EPS)
            inv = row_tile()
            nc.vector.reciprocal(out=inv[:], in_=norm[:])

            mh, sh, ch = row_tile(), row_tile(), row_tile()
            nc.vector.tensor_tensor(out=mh[:], in0=mean[:], in1=inv[:],
                                    op=ALU.mult)
            nc.vector.tensor_tensor(out=sh[:], in0=std[:], in1=inv[:],
                                    op=ALU.mult)
            nc.vector.tensor_tensor(out=ch[:], in0=cr, in1=inv[:],
                                    op=ALU.mult)

            sm = singles.tile([1, 4], F32)
            nc.vector.reduce_sum(out=sm[:, 0:1], in_=mh[:],
                                 axis=mybir.AxisListType.X)
            nc.vector.reduce_sum(out=sm[:, 1:2], in_=sh[:],
                                 axis=mybir.AxisListType.X)
            nc.vector.reduce_sum(out=sm[:, 2:3], in_=ch[:],
                                 axis=mybir.AxisListType.X)

            acc = row_tile()
            nc.vector.tensor_scalar_mul(out=acc[:], in0=mh[:],
                                        scalar1=sm[:, 0:1])
            nc.vector.scalar_tensor_tensor(
                out=acc[:], in0=sh[:], scalar=sm[:, 1:2], in1=acc[:],
                op0=ALU.mult, op1=ALU.add)
            nc.vector.scalar_tensor_tensor(
                out=acc[:], in0=ch[:], scalar=sm[:, 2:3], in1=acc[:],
                op0=ALU.mult, op1=ALU.add)
            # acc = avg * N;  d = (acc - sum(acc)/N) / N = avg - mean(avg)
            ravg = singles.tile([1, 1], F32)
            nc.vector.reduce_sum(out=ravg[:], in_=acc[:],
                                 axis=mybir.AxisListType.X)
            rm = singles.tile([1, 1], F32)
            nc.vector.tensor_scalar_mul(out=rm[:], in0=ravg[:],
                                        scalar1=1.0 / float(N))
            d = row_tile()
            nc.vector.tensor_scalar(out=d[:], in0=acc[:], scalar1=rm[:],
                                    scalar2=1.0 / float(N),
                                    op0=ALU.subtract, op1=ALU.mult)
            res = row_tile()
            nc.scalar.activation(res[:], d[:], AF.Sigmoid)
            nc.sync.dma_start(out[:, 0:N], res[:])
            # debug ride-along: this core's raw [S1|S2|C] stats
            nc.sync.dma_start(out[:, N:N + 3 * NI], comb[:])
    nc.compile()
    return nc


_NC_CACHE = None


def _get_nc():
    global _NC_CACHE
    if _NC_CACHE is None:
        _NC_CACHE = build()
    return _NC_CACHE


def _in_maps(mask_logits):
    m = np.ascontiguousarray(np.asarray(mask_logits), dtype=np.float32)
    return [
        {"mask_logits": m[c * NI:(c + 1) * NI].reshape(NI, P, FD)}
        for c in range(NCORES)
    ]


def _run(mask_logits, trace=False):
    nc = _get_nc()
    res = bass_utils.run_bass_kernel_spmd(
        nc, _in_maps(mask_logits), core_ids=list(range(NCORES)), trace=trace)
    return res


def kernel(mask_logits):
    res = _run(mask_logits, trace=False)
    return res.results[0]["out"].reshape(-1)[:N].astype(np.float32)
